# revision 23
# baseline (speedup 1.0000x reference)
"""Trainium2 Bass kernel for nn_CustomModel_91259465106013.

VGG16 features (13 convs) on 16 images (two streams of 8), Laplacian
gradient energy at 5 scales per image, softmax over the 16 scalar scores.

Sharding: data-parallel over the 16 images, 2 per NeuronCore (8 cores).
Each core runs the full VGG + energy pipeline for its 2 images and returns
per-partition partial energy sums [128, 16] fp32; the host reduces those,
forms the 16 scores and applies the softmax.

Conv mapping: 3x3 conv as 9 shift-matmuls on the PE (contract over Cin on
partitions), bf16 operands with fp32 PSUM accumulation.  Feature maps are
stored zero-padded ([C, 2img, H+2, W+2], bf16) in DRAM and streamed in row
bands.  conv1_1 uses a host-built im2row (K=27).  Layers with K<128 use
tile_position row-group packing to run 2/4 matmuls concurrently.
"""

import math
import os
import sys
import time

import numpy as np
import ml_dtypes

sys.path.insert(0, "/opt/trn_rl_repo")

import concourse.bass as bass  # noqa: E402
import concourse.mybir as mybir  # noqa: E402
import concourse.tile as tile  # noqa: E402

BF16 = mybir.dt.bfloat16
F32 = mybir.dt.float32
NPBF16 = ml_dtypes.bfloat16
AOP = mybir.AluOpType
AFT = mybir.ActivationFunctionType

N_CORES = 8

# ---------------------------------------------------------------------------
# layer table
# ---------------------------------------------------------------------------
# (Cin, Cout, H) per conv layer; l0 consumes the host-built im2row (K=27).
# G = number of row-group packs (for contraction dim < 128).
# pool: emit fused 2x2 maxpool after this layer.
# energy: scale index (0..4) whose gradient energy is computed on this output.
_LAYER_DEFS = [
    dict(Cin=27, Cout=64, H=224, G=2, taps=1, pool=False, energy=None),
    dict(Cin=64, Cout=64, H=224, G=2, taps=9, pool=True, energy=0),
    dict(Cin=64, Cout=128, H=112, G=2, taps=9, pool=False, energy=None),
    dict(Cin=128, Cout=128, H=112, G=1, taps=9, pool=True, energy=1),
    dict(Cin=128, Cout=256, H=56, G=1, taps=9, pool=False, energy=None),
    dict(Cin=256, Cout=256, H=56, G=1, taps=9, pool=False, energy=None),
    dict(Cin=256, Cout=256, H=56, G=1, taps=9, pool=True, energy=2),
    dict(Cin=256, Cout=512, H=28, G=1, taps=9, pool=False, energy=None),
    dict(Cin=512, Cout=512, H=28, G=1, taps=9, pool=False, energy=None),
    dict(Cin=512, Cout=512, H=28, G=1, taps=9, pool=True, energy=3),
    dict(Cin=512, Cout=512, H=14, G=1, taps=9, pool=False, energy=None),
    dict(Cin=512, Cout=512, H=14, G=1, taps=9, pool=False, energy=None),
    dict(Cin=512, Cout=512, H=14, G=1, taps=9, pool=False, energy=4),
]

_R_BY_H = {224: 1, 112: 2, 56: 4, 28: 7, 14: 14}     # output rows per psum chunk
_RO_BY_H = {224: 16, 112: 28, 56: 56, 28: 28, 14: 14}  # output rows per band
_RE_BY_H = {224: 8, 112: 8, 56: 14, 28: 14, 14: 14}  # energy band rows

_SCALE_C = [64, 128, 256, 512, 512]
_SCALE_H = [224, 112, 56, 28, 14]
# result column map: scale 0 is image-packed on partitions (one column);
# scales 1..4 use two columns (img0, img1).
_SCALE_COLS = {0: (0,), 1: (1, 2), 2: (3, 4), 3: (5, 6), 4: (7, 8)}


def _layer_meta():
    layers = []
    col = 0
    bcol = 0
    for ld in _LAYER_DEFS:
        L = dict(ld)
        Cin, Cout, H = L["Cin"], L["Cout"], L["H"]
        L["W"] = H
        L["ci_tiles"] = max(1, Cin // 128)
        L["co_tiles"] = max(1, Cout // 128)
        L["M"] = min(128, Cout)
        L["K"] = Cin if Cin <= 128 else 128
        L["R"] = _R_BY_H[H]
        L["RO"] = _RO_BY_H[H]
        L["wcol"] = col
        col += L["ci_tiles"] * L["taps"] * Cout
        L["bcol"] = bcol
        bcol += L["co_tiles"]
        layers.append(L)
    return layers, col, bcol


LAYERS, WB_COLS, BB_COLS = _layer_meta()


# ---------------------------------------------------------------------------
# bass program
# ---------------------------------------------------------------------------

def _emit_conv_layer(tc, nc, li, L, src, dst, pooled, wb, bias_sb, zrow):
    """One conv layer: src fmap (DRAM, padded bf16) -> dst fmap, optional
    fused 2x2 maxpool into `pooled`."""
    import contextlib

    Cin, Cout, H, W = L["Cin"], L["Cout"], L["H"], L["W"]
    G, taps, M, K = L["G"], L["taps"], L["M"], L["K"]
    R, RO = L["R"], L["RO"]
    ci_tiles, co_tiles = L["ci_tiles"], L["co_tiles"]
    Hp, Wp = H + 2, W + 2
    im2row = taps == 1
    chunks = RO // R                  # psum chunks per band
    P_CH = G if G > 1 else min(2, chunks)   # chunks per psum tile
    GK = 32 if L["K"] < 64 else 64    # partition stride between row groups
    n_acc = ci_tiles * taps           # matmuls accumulated per chunk
    nrw = 2 * R * W                   # psum free elements per chunk

    in_rows = RO if im2row else RO + 2

    n_bands = H // RO
    if n_bands == 1:
        in_bufs = ci_tiles
    elif H == 224:
        in_bufs = 3
    else:
        in_bufs = 2 * ci_tiles
    out_bufs = 2 if H == 224 else max(2, co_tiles + 1)
    with contextlib.ExitStack() as ctx:
        inpool = ctx.enter_context(
            tc.tile_pool(name=f"in{li}", bufs=in_bufs))
        outpool = ctx.enter_context(
            tc.tile_pool(name=f"out{li}", bufs=out_bufs))
        pspool = ctx.enter_context(
            tc.tile_pool(name=f"ps{li}", bufs=min(3, 8 // P_CH), space="PSUM"))
        plpool = None
        if L["pool"]:
            plpool = ctx.enter_context(
                tc.tile_pool(name=f"pl{li}", bufs=1 if H == 224 else 2))

        # layer weights (per ci tile), resident for the whole layer
        wts = []
        for ci in range(ci_tiles):
            wt = tc.wpool.tile([128, taps * Cout], BF16, tag="w")
            c0 = L["wcol"] + ci * taps * Cout
            nc.sync.dma_start(wt, wb[:, c0:c0 + taps * Cout])
            wts.append(wt)

        # zero top/bottom padding rows of dst (and pooled) once
        for co in range(co_tiles):
            cs = co * 128
            nc.sync.dma_start(dst[cs:cs + M, :, 0, :], zrow[0:M, :, 0:Wp])
            nc.sync.dma_start(dst[cs:cs + M, :, Hp - 1, :], zrow[0:M, :, 0:Wp])
            if L["pool"]:
                W2p = W // 2 + 2
                nc.sync.dma_start(pooled[cs:cs + M, :, 0, :], zrow[0:M, :, 0:W2p])
                nc.sync.dma_start(pooled[cs:cs + M, :, H // 2 + 1, :], zrow[0:M, :, 0:W2p])

        for y0 in range(0, H, RO):
            # ---- load input band (padded rows y0 .. y0+RO+1) ----
            bands = []
            for ci in range(ci_tiles):
                it = inpool.tile([128, 2, in_rows, W if im2row else Wp], BF16,
                                 tag=f"inb{ci}")
                if im2row:
                    # group 0 from DRAM, replicas via SBUF-to-SBUF copy
                    nc.sync.dma_start(it[0:K], src[0:K, :, y0:y0 + RO, :])
                    for g in range(1, G):
                        nc.sync.dma_start(it[g * GK:g * GK + K], it[0:K])
                elif G > 1:
                    nc.sync.dma_start(it[0:K], src[0:K, :, y0:y0 + RO + 2, :])
                    for g in range(1, G):
                        nc.sync.dma_start(it[g * GK:g * GK + K], it[0:K])
                else:
                    nc.sync.dma_start(it, src[ci * 128:ci * 128 + 128, :,
                                              y0:y0 + RO + 2, :])
                bands.append(it)

            for co in range(co_tiles):
                cs = co * 128
                ost = outpool.tile([M, 2, RO, Wp], BF16, tag="ost")
                # zero the left/right padding columns
                nc.gpsimd.memset(ost[:, :, :, 0], 0.0)
                nc.gpsimd.memset(ost[:, :, :, Wp - 1], 0.0)

                for cb0 in range(0, chunks, P_CH):
                    ps = pspool.tile([M, P_CH, 512], F32, tag="ps")
                    for ci in range(ci_tiles):
                        for tap in range(taps):
                            dyi, dxi = divmod(tap, 3)
                            lcol = tap * Cout + cs
                            for c in range(P_CH):
                                rb = cb0 + c
                                g = c if G > 1 else 0
                                gb = g * GK
                                lhsT = wts[ci][gb:gb + K, lcol:lcol + M]
                                band = bands[ci]
                                if im2row:
                                    rhs = band[gb:gb + K, :, rb, :]
                                else:
                                    r0 = rb * R + dyi
                                    rhs = band[gb:gb + K, :, r0:r0 + R,
                                               dxi:dxi + W]
                                out_c = ps[:, c, :nrw].rearrange(
                                    "m (i r w) -> m i r w", i=2, r=R)
                                idx = ci * taps + tap
                                nc.tensor.matmul(
                                    out_c, lhsT, rhs,
                                    start=(idx == 0), stop=(idx == n_acc - 1),
                                    tile_position=(gb, 0) if G > 1 else None)
                    # bias + relu, one act per chunk (ACT APs max 3 free dims)
                    npch = min(P_CH, chunks - cb0)
                    out5 = ost.rearrange("m i (c r) w -> m c i r w", r=R)
                    for c in range(npch):
                        ps_in = ps[:, c, :nrw].rearrange(
                            "m (i r w) -> m i r w", i=2, r=R)
                        out_ap = out5[:, cb0 + c, :, :, 1:1 + W]
                        nc.scalar.activation(
                            out_ap, ps_in, AFT.Relu,
                            bias=bias_sb[0:M, L["bcol"] + co:L["bcol"] + co + 1])

                # ---- store band ----
                nc.sync.dma_start(dst[cs:cs + M, :, y0 + 1:y0 + 1 + RO, :], ost)

                if L["pool"]:
                    W2 = W // 2
                    W2p = W2 + 2
                    pm1 = plpool.tile([M, 2, RO, W2], BF16, tag="pm1")
                    cin = ost[:, :, :, 1:1 + W].rearrange(
                        "m i r (x t) -> m i r x t", t=2)
                    nc.vector.tensor_max(pm1, cin[:, :, :, :, 0],
                                         cin[:, :, :, :, 1])
                    pst = plpool.tile([M, 2, RO // 2, W2p], BF16, tag="pst")
                    nc.gpsimd.memset(pst[:, :, :, 0], 0.0)
                    nc.gpsimd.memset(pst[:, :, :, W2p - 1], 0.0)
                    rin = pm1.rearrange("m i (p t) x -> m i p t x", t=2)
                    nc.vector.tensor_max(pst[:, :, :, 1:1 + W2],
                                         rin[:, :, :, 0, :], rin[:, :, :, 1, :])
                    nc.sync.dma_start(
                        pooled[cs:cs + M, :, y0 // 2 + 1:y0 // 2 + 1 + RO // 2, :],
                        pst)


def _emit_energy(tc, nc, si, F, res, ep):
    """Gradient-energy pass for scale si over padded fmap F (DRAM, bf16).

    u = boxsum3x3(F) - 9*F = 8*g ; accumulates sum(u^2) per partition into
    res columns (host divides by 64*C*H*W).  Uses the top-level co-resident
    pools in `ep` so this work overlaps the following conv layers.  Row sums
    run on GpSimd (otherwise idle) to unload the DVE."""
    C = _SCALE_C[si]
    H = W = _SCALE_H[si]
    RE = _RE_BY_H[H]
    cols = _SCALE_COLS[si]
    packed = C == 64
    ci_tiles = max(1, C // 128)
    Wp = W + 2

    for ci in range(ci_tiles):
        cs = ci * 128
        for y0 in range(0, H, RE):
            if packed:
                # both images on partitions: p<64 img0, p>=64 img1
                ft = ep["f"].tile([128, RE + 2, Wp], BF16, tag="ft")
                nc.sync.dma_start(ft[0:64], F[0:64, 0, y0:y0 + RE + 2, :])
                nc.sync.dma_start(ft[64:128], F[0:64, 1, y0:y0 + RE + 2, :])
                r = ep["r"].tile([128, RE + 2, W], BF16, tag="r")
                nc.gpsimd.tensor_add(r, ft[:, :, 0:W], ft[:, :, 2:W + 2])
                nc.vector.tensor_add(r, r, ft[:, :, 1:W + 1])
                t1 = ep["t"].tile([128, RE, W], BF16, tag="t1")
                nc.vector.tensor_add(t1, r[:, 0:RE], r[:, 2:RE + 2])
                nc.vector.tensor_add(t1, t1, r[:, 1:RE + 1])
                nc.vector.scalar_tensor_tensor(
                    t1, ft[:, 1:RE + 1, 1:W + 1], -9.0, t1,
                    AOP.mult, AOP.add)
                sq = ep["s"].tile([128, RE, W], BF16, tag="sq")
                pacc = ep["a"].tile([128, 1], F32, tag="pacc")
                nc.scalar.activation(sq, t1, AFT.Square, accum_out=pacc)
                c0 = cols[0]
                nc.vector.tensor_add(res[:, c0:c0 + 1], res[:, c0:c0 + 1], pacc)
            else:
                ft = ep["f"].tile([128, 2, RE + 2, Wp], BF16, tag="ft")
                nc.sync.dma_start(ft, F[cs:cs + 128, :, y0:y0 + RE + 2, :])
                r = ep["r"].tile([128, 2, RE + 2, W], BF16, tag="r")
                nc.gpsimd.tensor_add(r, ft[:, :, :, 0:W], ft[:, :, :, 2:W + 2])
                nc.vector.tensor_add(r, r, ft[:, :, :, 1:W + 1])
                t1 = ep["t"].tile([128, 2, RE, W], BF16, tag="t1")
                nc.vector.tensor_add(t1, r[:, :, 0:RE], r[:, :, 2:RE + 2])
                nc.vector.tensor_add(t1, t1, r[:, :, 1:RE + 1])
                for img in range(2):
                    # ScalarTensorTensor is limited to 3D access patterns
                    nc.vector.scalar_tensor_tensor(
                        t1[:, img], ft[:, img, 1:RE + 1, 1:W + 1], -9.0,
                        t1[:, img], AOP.mult, AOP.add)
                for img in range(2):
                    sq = ep["s"].tile([128, RE, W], BF16, tag="sq")
                    pacc = ep["a"].tile([128, 1], F32, tag="pacc")
                    nc.scalar.activation(sq, t1[:, img], AFT.Square,
                                         accum_out=pacc)
                    c0 = cols[img]
                    nc.vector.tensor_add(res[:, c0:c0 + 1],
                                         res[:, c0:c0 + 1], pacc)


_MAX_WAITS = 1


def _split_excess_waits(nc, lim=_MAX_WAITS):
    """Hardware engine instructions have a bounded number of sync-wait slots.
    Tile can attach more after wide cross-engine fan-ins (walrus then fails
    with 'Too many sync wait commands').  Move excess waits onto same-engine
    NOPs inserted immediately before the offending instruction — queue order
    on the engine preserves semantics."""
    nid = [0]
    for bb in nc.main_func.blocks:
        insts = list(bb.instructions)
        out = []
        changed = False
        for ins in insts:
            si = ins.sync_info
            ilim = 2 if isinstance(ins, mybir.InstEventSemaphore) else lim
            if si is not None and si.on_wait and len(si.on_wait) > ilim:
                waits = list(si.on_wait)
                head, tail = waits[:-ilim], waits[-ilim:]
                while head:
                    chunk, head = head[:lim], head[lim:]
                    nop = mybir.InstNoOp(name=f"waitnop-{nid[0]}", ins=[],
                                         outs=[], engine=ins.engine)
                    nid[0] += 1
                    nop.sync_info = mybir.SyncInfo(on_wait=chunk, on_update=[])
                    out.append(nop)
                ins.sync_info = mybir.SyncInfo(on_wait=tail,
                                              on_update=list(si.on_update))
                changed = True
            out.append(ins)
        if changed:
            bb.instructions = out


def build_nc():
    nc = bass.Bass()

    x0 = nc.dram_tensor("x0", [27, 2, 224, 224], BF16, kind="ExternalInput")
    wb = nc.dram_tensor("wb", [128, WB_COLS], BF16, kind="ExternalInput")
    bb = nc.dram_tensor("bb", [128, BB_COLS], F32, kind="ExternalInput")
    eout = nc.dram_tensor("eout", [128, 16], F32, kind="ExternalOutput")

    # DRAM scratch fmaps (padded, bf16)
    fmaps = []          # conv outputs per layer
    pooled = {}         # pooled outputs per pooling layer index
    for li, L in enumerate(LAYERS):
        H = L["H"]
        fmaps.append(nc.dram_tensor(f"f{li}", [L["Cout"], 2, H + 2, H + 2], BF16))
        if L["pool"]:
            pooled[li] = nc.dram_tensor(f"p{li}",
                                        [L["Cout"], 2, H // 2 + 2, H // 2 + 2],
                                        BF16)

    with tile.TileContext(nc) as tc:
        import contextlib
        with contextlib.ExitStack() as top:
            tc.wpool = top.enter_context(tc.tile_pool(name="w", bufs=5))
            const_pool = top.enter_context(tc.tile_pool(name="const", bufs=1))
            # top-level co-resident energy pools: energy work overlaps the
            # conv layers that follow its producing layer
            ep = {
                "f": top.enter_context(tc.tile_pool(name="ef", bufs=2)),
                "r": top.enter_context(tc.tile_pool(name="er", bufs=2)),
                "t": top.enter_context(tc.tile_pool(name="et", bufs=2)),
                "s": top.enter_context(tc.tile_pool(name="es", bufs=1)),
                "a": top.enter_context(tc.tile_pool(name="ea", bufs=4)),
            }

            bias_sb = const_pool.tile([128, BB_COLS], F32)
            nc.sync.dma_start(bias_sb, bb[:, :])
            zrow = const_pool.tile([128, 2, 226], BF16)
            nc.gpsimd.memset(zrow, 0.0)
            res = const_pool.tile([128, 16], F32)
            nc.vector.memset(res, 0.0)

            src = x0
            energy_fmaps = {}
            for li, L in enumerate(LAYERS):
                dst = fmaps[li]
                pl = pooled.get(li)
                with nc.named_scope(f"conv{li}"):
                    _emit_conv_layer(tc, nc, li, L, src, dst, pl, wb, bias_sb,
                                     zrow)
                if L["energy"] is not None:
                    energy_fmaps[L["energy"]] = dst
                src = pl if L["pool"] else dst

            # energy passes emitted LAST: lower scheduler priority, so the
            # conv chain keeps the contended engines and energy fills idle
            # slots (each band is dep-ready as soon as its fmap rows land)
            for si in range(5):
                with nc.named_scope(f"energy{si}"):
                    _emit_energy(tc, nc, si, energy_fmaps[si], res, ep)

            nc.sync.dma_start(eout[:, :], res)

    _split_excess_waits(nc)
    return nc


# ---------------------------------------------------------------------------
# host-side input prep / output decode
# ---------------------------------------------------------------------------

def _prep_weights(params):
    WB = np.zeros((128, WB_COLS), np.float32)
    BB = np.zeros((128, BB_COLS), np.float32)
    for li, L in enumerate(LAYERS):
        w = np.asarray(params[li][0], np.float32)   # [Cout, Cin, 3, 3]
        b = np.asarray(params[li][1], np.float32)   # [Cout]
        Cout = L["Cout"]
        GK = 32 if L["K"] < 64 else 64
        if li == 0:
            # im2row: row = (3*dy+dx)*3 + c, single tap, replicated per group
            blk = np.transpose(w, (2, 3, 1, 0)).reshape(27, Cout)
            for g in range(L["G"]):
                WB[g * GK:g * GK + 27, L["wcol"]:L["wcol"] + Cout] = blk
        else:
            Cin, taps = L["Cin"], L["taps"]
            for ci in range(L["ci_tiles"]):
                K = min(128, Cin - ci * 128)
                blk = np.transpose(w[:, ci * 128:ci * 128 + K], (1, 2, 3, 0))
                blk = blk.reshape(K, taps * Cout)  # [k, tap*Cout]
                c0 = L["wcol"] + ci * taps * Cout
                if L["G"] > 1:
                    for g in range(L["G"]):
                        WB[g * GK:g * GK + K, c0:c0 + taps * Cout] = blk
                else:
                    WB[0:K, c0:c0 + taps * Cout] = blk
        for co in range(L["co_tiles"]):
            M = min(128, Cout - co * 128)
            BB[0:M, L["bcol"] + co] = b[co * 128:co * 128 + M]
    return WB.astype(NPBF16), BB


def _prep_im2row(img_pair):
    """img_pair: [2, 3, 224, 224] fp32 -> [27, 2, 224, 224] bf16."""
    xp = np.zeros((2, 3, 226, 226), np.float32)
    xp[:, :, 1:225, 1:225] = img_pair
    A = np.empty((27, 2, 224, 224), np.float32)
    for t in range(9):
        dy, dx = divmod(t, 3)
        for c in range(3):
            A[t * 3 + c] = xp[:, c, dy:dy + 224, dx:dx + 224]
    return A.astype(NPBF16)


def _decode(res_list):
    """res_list: per-core [128, 16] fp32 -> softmax over all 16 images."""
    scores = np.zeros(16, np.float64)
    for k, res in enumerate(res_list):
        res = np.asarray(res, np.float64)
        for i_loc in range(2):
            gimg = 2 * k + i_loc
            e = []
            for si in range(5):
                C, H = _SCALE_C[si], _SCALE_H[si]
                cols = _SCALE_COLS[si]
                if len(cols) == 1:
                    s = res[64 * i_loc:64 * (i_loc + 1), cols[0]].sum()
                else:
                    s = res[:, cols[i_loc]].sum()
                e.append(s / (64.0 * C * H * H))
            scores[gimg] = np.mean(e)
    # image order: first 8 are S1, next 8 are S2 (cores hold [2k, 2k+1])
    z = scores - scores.max()
    p = np.exp(z)
    p /= p.sum()
    return p.astype(np.float32)


_NC_CACHE = {}
LAST_RESULT = None


def kernel(S1_VGG_in, S2_VGG_in, params):
    global LAST_RESULT
    from concourse.bass_utils import run_bass_kernel_spmd

    S1 = np.asarray(S1_VGG_in, np.float32)
    S2 = np.asarray(S2_VGG_in, np.float32)
    imgs = np.concatenate([S1, S2], axis=0)      # [16, 3, 224, 224]

    WB, BB = _prep_weights(params)
    in_maps = []
    for k in range(N_CORES):
        x0 = _prep_im2row(imgs[2 * k:2 * k + 2])
        in_maps.append({"x0": x0, "wb": WB, "bb": BB})

    if "nc" not in _NC_CACHE:
        _NC_CACHE["nc"] = build_nc()
    nc = _NC_CACHE["nc"]

    trace = os.environ.get("KBENCH_TRACE", "0") == "1"
    kw = {}
    if trace:
        kw = dict(trace=True, trace_cores=[0])
    r = run_bass_kernel_spmd(nc, in_maps, core_ids=list(range(N_CORES)), **kw)
    LAST_RESULT = r
    return _decode([m["eout"] for m in r.results])


if __name__ == "__main__":
    if "--dry" in sys.argv:
        t0 = time.time()
        nc = build_nc()
        n = len(nc.m.functions[0].blocks[0].instructions) if hasattr(
            nc.m.functions[0], "blocks") else -1
        print(f"build ok in {time.time()-t0:.1f}s")
        try:
            total = sum(len(bb.instructions) for bb in nc.main_func.blocks)
            print("instructions:", total)
        except Exception as e:
            print("count failed:", e)


# revision 29
# speedup vs baseline: 1.0177x; 1.0177x over previous
"""Trainium2 Bass kernel for nn_CustomModel_91259465106013.

VGG16 features (13 convs) on 16 images (two streams of 8), Laplacian
gradient energy at 5 scales per image, softmax over the 16 scalar scores.

Sharding: data-parallel over the 16 images, 2 per NeuronCore (8 cores).
Each core runs the full VGG + energy pipeline for its 2 images and returns
per-partition partial energy sums [128, 16] fp32; the host reduces those,
forms the 16 scores and applies the softmax.

Conv mapping: 3x3 conv as 9 shift-matmuls on the PE (contract over Cin on
partitions), bf16 operands with fp32 PSUM accumulation.  Feature maps are
stored zero-padded ([C, 2img, H+2, W+2], bf16) in DRAM and streamed in row
bands.  conv1_1 uses a host-built im2row (K=27).  Layers with K<128 use
tile_position row-group packing to run 2/4 matmuls concurrently.
"""

import math
import os
import sys
import time

import numpy as np
import ml_dtypes

sys.path.insert(0, "/opt/trn_rl_repo")

import concourse.bass as bass  # noqa: E402
import concourse.mybir as mybir  # noqa: E402
import concourse.tile as tile  # noqa: E402

BF16 = mybir.dt.bfloat16
F32 = mybir.dt.float32
NPBF16 = ml_dtypes.bfloat16
AOP = mybir.AluOpType
AFT = mybir.ActivationFunctionType

N_CORES = 8

# ---------------------------------------------------------------------------
# layer table
# ---------------------------------------------------------------------------
# (Cin, Cout, H) per conv layer; l0 consumes the host-built im2row (K=27).
# G = number of row-group packs (for contraction dim < 128).
# pool: emit fused 2x2 maxpool after this layer.
# energy: scale index (0..4) whose gradient energy is computed on this output.
_LAYER_DEFS = [
    dict(Cin=27, Cout=64, H=224, G=2, taps=1, pool=False, energy=None),
    dict(Cin=64, Cout=64, H=224, G=2, taps=9, pool=True, energy=0),
    dict(Cin=64, Cout=128, H=112, G=2, taps=9, pool=False, energy=None),
    dict(Cin=128, Cout=128, H=112, G=1, taps=9, pool=True, energy=1),
    dict(Cin=128, Cout=256, H=56, G=1, taps=9, pool=False, energy=None),
    dict(Cin=256, Cout=256, H=56, G=1, taps=9, pool=False, energy=None),
    dict(Cin=256, Cout=256, H=56, G=1, taps=9, pool=True, energy=2),
    dict(Cin=256, Cout=512, H=28, G=1, taps=9, pool=False, energy=None),
    dict(Cin=512, Cout=512, H=28, G=1, taps=9, pool=False, energy=None),
    dict(Cin=512, Cout=512, H=28, G=1, taps=9, pool=True, energy=3),
    dict(Cin=512, Cout=512, H=14, G=1, taps=9, pool=False, energy=None),
    dict(Cin=512, Cout=512, H=14, G=1, taps=9, pool=False, energy=None),
    dict(Cin=512, Cout=512, H=14, G=1, taps=9, pool=False, energy=4),
]

_R_BY_H = {224: 1, 112: 2, 56: 4, 28: 7, 14: 14}     # output rows per psum chunk
_RO_BY_H = {224: 16, 112: 28, 56: 56, 28: 28, 14: 14}  # output rows per band
_RE_BY_H = {224: 8, 112: 8, 56: 14, 28: 14, 14: 14}  # energy band rows

_SCALE_C = [64, 128, 256, 512, 512]
_SCALE_H = [224, 112, 56, 28, 14]
# result column map: scale 0 is image-packed on partitions (one column);
# scales 1..4 use two columns (img0, img1).
_SCALE_COLS = {0: (0,), 1: (1, 2), 2: (3, 4), 3: (5, 6), 4: (7, 8)}


def _layer_meta():
    layers = []
    col = 0
    bcol = 0
    for ld in _LAYER_DEFS:
        L = dict(ld)
        Cin, Cout, H = L["Cin"], L["Cout"], L["H"]
        L["W"] = H
        L["ci_tiles"] = max(1, Cin // 128)
        L["co_tiles"] = max(1, Cout // 128)
        L["M"] = min(128, Cout)
        L["K"] = Cin if Cin <= 128 else 128
        L["R"] = _R_BY_H[H]
        L["RO"] = _RO_BY_H[H]
        L["wcol"] = col
        col += L["ci_tiles"] * L["taps"] * Cout
        L["bcol"] = bcol
        bcol += L["co_tiles"]
        layers.append(L)
    return layers, col, bcol


LAYERS, WB_COLS, BB_COLS = _layer_meta()


# ---------------------------------------------------------------------------
# bass program
# ---------------------------------------------------------------------------

def _emit_conv_layer(tc, nc, li, L, src, dst, pooled, wb, bias_sb, zrow):
    """One conv layer: src fmap (DRAM, padded bf16) -> dst fmap, optional
    fused 2x2 maxpool into `pooled`."""
    import contextlib

    Cin, Cout, H, W = L["Cin"], L["Cout"], L["H"], L["W"]
    G, taps, M, K = L["G"], L["taps"], L["M"], L["K"]
    R, RO = L["R"], L["RO"]
    ci_tiles, co_tiles = L["ci_tiles"], L["co_tiles"]
    Hp, Wp = H + 2, W + 2
    im2row = taps == 1
    chunks = RO // R                  # psum chunks per band
    P_CH = G if G > 1 else min(2, chunks)   # chunks per psum tile
    GK = 32 if L["K"] < 64 else 64    # partition stride between row groups
    n_acc = ci_tiles * taps           # matmuls accumulated per chunk
    nrw = 2 * R * W                   # psum free elements per chunk

    in_rows = RO if im2row else RO + 2

    n_bands = H // RO
    if n_bands == 1:
        in_bufs = ci_tiles
    elif H == 224:
        in_bufs = 3
    else:
        in_bufs = 2 * ci_tiles
    out_bufs = 2 if H == 224 else max(2, co_tiles + 1)
    with contextlib.ExitStack() as ctx:
        inpool = ctx.enter_context(
            tc.tile_pool(name=f"in{li}", bufs=in_bufs))
        outpool = ctx.enter_context(
            tc.tile_pool(name=f"out{li}", bufs=out_bufs))
        pspool = ctx.enter_context(
            tc.tile_pool(name=f"ps{li}", bufs=min(3, 8 // P_CH), space="PSUM"))
        plpool = None
        if L["pool"]:
            plpool = ctx.enter_context(
                tc.tile_pool(name=f"pl{li}", bufs=1 if H == 224 else 2))

        # layer weights (per ci tile), resident for the whole layer
        wts = []
        for ci in range(ci_tiles):
            wt = tc.wpool.tile([128, taps * Cout], BF16, tag="w")
            c0 = L["wcol"] + ci * taps * Cout
            nc.sync.dma_start(wt, wb[:, c0:c0 + taps * Cout])
            wts.append(wt)

        # zero top/bottom padding rows of dst (and pooled) once
        for co in range(co_tiles):
            cs = co * 128
            nc.sync.dma_start(dst[cs:cs + M, :, 0, :], zrow[0:M, :, 0:Wp])
            nc.sync.dma_start(dst[cs:cs + M, :, Hp - 1, :], zrow[0:M, :, 0:Wp])
            if L["pool"]:
                W2p = W // 2 + 2
                nc.sync.dma_start(pooled[cs:cs + M, :, 0, :], zrow[0:M, :, 0:W2p])
                nc.sync.dma_start(pooled[cs:cs + M, :, H // 2 + 1, :], zrow[0:M, :, 0:W2p])

        for y0 in range(0, H, RO):
            # ---- load input band (padded rows y0 .. y0+RO+1) ----
            bands = []
            for ci in range(ci_tiles):
                it = inpool.tile([128, 2, in_rows, W if im2row else Wp], BF16,
                                 tag=f"inb{ci}")
                if im2row:
                    # group 0 from DRAM, replicas via SBUF-to-SBUF copy
                    nc.sync.dma_start(it[0:K], src[0:K, :, y0:y0 + RO, :])
                    for g in range(1, G):
                        nc.sync.dma_start(it[g * GK:g * GK + K], it[0:K])
                elif G > 1:
                    nc.sync.dma_start(it[0:K], src[0:K, :, y0:y0 + RO + 2, :])
                    for g in range(1, G):
                        nc.sync.dma_start(it[g * GK:g * GK + K], it[0:K])
                else:
                    nc.sync.dma_start(it, src[ci * 128:ci * 128 + 128, :,
                                              y0:y0 + RO + 2, :])
                bands.append(it)

            for co in range(co_tiles):
                cs = co * 128
                ost = outpool.tile([M, 2, RO, Wp], BF16, tag="ost")
                # zero the left/right padding columns
                nc.gpsimd.memset(ost[:, :, :, 0], 0.0)
                nc.gpsimd.memset(ost[:, :, :, Wp - 1], 0.0)

                for cb0 in range(0, chunks, P_CH):
                    ps = pspool.tile([M, P_CH, 512], F32, tag="ps")
                    for ci in range(ci_tiles):
                        for tap in range(taps):
                            dyi, dxi = divmod(tap, 3)
                            lcol = tap * Cout + cs
                            for c in range(P_CH):
                                rb = cb0 + c
                                g = c if G > 1 else 0
                                gb = g * GK
                                lhsT = wts[ci][gb:gb + K, lcol:lcol + M]
                                band = bands[ci]
                                if im2row:
                                    rhs = band[gb:gb + K, :, rb, :]
                                else:
                                    r0 = rb * R + dyi
                                    rhs = band[gb:gb + K, :, r0:r0 + R,
                                               dxi:dxi + W]
                                out_c = ps[:, c, :nrw].rearrange(
                                    "m (i r w) -> m i r w", i=2, r=R)
                                idx = ci * taps + tap
                                nc.tensor.matmul(
                                    out_c, lhsT, rhs,
                                    start=(idx == 0), stop=(idx == n_acc - 1),
                                    tile_position=(gb, 0) if G > 1 else None)
                    # bias + relu, one act per chunk (ACT APs max 3 free dims).
                    # The 64-partition H=224 layers are ACT-throughput-bound:
                    # route every other chunk through the (otherwise idle) DVE.
                    npch = min(P_CH, chunks - cb0)
                    out5 = ost.rearrange("m i (c r) w -> m c i r w", r=R)
                    bias_ap = bias_sb[0:M, L["bcol"] + co:L["bcol"] + co + 1]
                    for c in range(npch):
                        ps_in = ps[:, c, :nrw].rearrange(
                            "m (i r w) -> m i r w", i=2, r=R)
                        out_ap = out5[:, cb0 + c, :, :, 1:1 + W]
                        if H == 224 and c % 2 == 1 and _ZERO_BIAS[0]:
                            # plain relu on the DVE (biases are all zero for
                            # this problem; PSUM-src + AP-scalar hangs the HW)
                            nc.vector.tensor_scalar_max(out_ap, ps_in, 0.0)
                        else:
                            nc.scalar.activation(out_ap, ps_in, AFT.Relu,
                                                 bias=bias_ap)

                # ---- store band ----
                nc.sync.dma_start(dst[cs:cs + M, :, y0 + 1:y0 + 1 + RO, :], ost)

                if L["pool"]:
                    W2 = W // 2
                    W2p = W2 + 2
                    pm1 = plpool.tile([M, 2, RO, W2], BF16, tag="pm1")
                    cin = ost[:, :, :, 1:1 + W].rearrange(
                        "m i r (x t) -> m i r x t", t=2)
                    nc.vector.tensor_max(pm1, cin[:, :, :, :, 0],
                                         cin[:, :, :, :, 1])
                    pst = plpool.tile([M, 2, RO // 2, W2p], BF16, tag="pst")
                    nc.gpsimd.memset(pst[:, :, :, 0], 0.0)
                    nc.gpsimd.memset(pst[:, :, :, W2p - 1], 0.0)
                    rin = pm1.rearrange("m i (p t) x -> m i p t x", t=2)
                    nc.vector.tensor_max(pst[:, :, :, 1:1 + W2],
                                         rin[:, :, :, 0, :], rin[:, :, :, 1, :])
                    nc.sync.dma_start(
                        pooled[cs:cs + M, :, y0 // 2 + 1:y0 // 2 + 1 + RO // 2, :],
                        pst)


def _emit_energy(tc, nc, si, F, res, ep):
    """Gradient-energy pass for scale si over padded fmap F (DRAM, bf16).

    u = boxsum3x3(F) - 9*F = 8*g ; accumulates sum(u^2) per partition into
    res columns (host divides by 64*C*H*W).  Uses the top-level co-resident
    pools in `ep` so this work overlaps the following conv layers.  Row sums
    run on GpSimd (otherwise idle) to unload the DVE."""
    C = _SCALE_C[si]
    H = W = _SCALE_H[si]
    RE = _RE_BY_H[H]
    cols = _SCALE_COLS[si]
    packed = C == 64
    ci_tiles = max(1, C // 128)
    Wp = W + 2

    for ci in range(ci_tiles):
        cs = ci * 128
        for y0 in range(0, H, RE):
            if packed:
                # both images on partitions: p<64 img0, p>=64 img1
                ft = ep["f"].tile([128, RE + 2, Wp], BF16, tag="ft")
                nc.sync.dma_start(ft[0:64], F[0:64, 0, y0:y0 + RE + 2, :])
                nc.sync.dma_start(ft[64:128], F[0:64, 1, y0:y0 + RE + 2, :])
                r = ep["r"].tile([128, RE + 2, W], BF16, tag="r")
                nc.gpsimd.tensor_add(r, ft[:, :, 0:W], ft[:, :, 2:W + 2])
                nc.vector.tensor_add(r, r, ft[:, :, 1:W + 1])
                t1 = ep["t"].tile([128, RE, W], BF16, tag="t1")
                nc.vector.tensor_add(t1, r[:, 0:RE], r[:, 2:RE + 2])
                nc.vector.tensor_add(t1, t1, r[:, 1:RE + 1])
                nc.vector.scalar_tensor_tensor(
                    t1, ft[:, 1:RE + 1, 1:W + 1], -9.0, t1,
                    AOP.mult, AOP.add)
                sq = ep["s"].tile([128, RE, W], BF16, tag="sq")
                pacc = ep["a"].tile([128, 1], F32, tag="pacc")
                nc.scalar.activation(sq, t1, AFT.Square, accum_out=pacc)
                c0 = cols[0]
                nc.vector.tensor_add(res[:, c0:c0 + 1], res[:, c0:c0 + 1], pacc)
            else:
                ft = ep["f"].tile([128, 2, RE + 2, Wp], BF16, tag="ft")
                nc.sync.dma_start(ft, F[cs:cs + 128, :, y0:y0 + RE + 2, :])
                r = ep["r"].tile([128, 2, RE + 2, W], BF16, tag="r")
                nc.gpsimd.tensor_add(r, ft[:, :, :, 0:W], ft[:, :, :, 2:W + 2])
                nc.vector.tensor_add(r, r, ft[:, :, :, 1:W + 1])
                t1 = ep["t"].tile([128, 2, RE, W], BF16, tag="t1")
                nc.vector.tensor_add(t1, r[:, :, 0:RE], r[:, :, 2:RE + 2])
                nc.vector.tensor_add(t1, t1, r[:, :, 1:RE + 1])
                for img in range(2):
                    # ScalarTensorTensor is limited to 3D access patterns
                    nc.vector.scalar_tensor_tensor(
                        t1[:, img], ft[:, img, 1:RE + 1, 1:W + 1], -9.0,
                        t1[:, img], AOP.mult, AOP.add)
                for img in range(2):
                    sq = ep["s"].tile([128, RE, W], BF16, tag="sq")
                    pacc = ep["a"].tile([128, 1], F32, tag="pacc")
                    nc.scalar.activation(sq, t1[:, img], AFT.Square,
                                         accum_out=pacc)
                    c0 = cols[img]
                    nc.vector.tensor_add(res[:, c0:c0 + 1],
                                         res[:, c0:c0 + 1], pacc)


_MAX_WAITS = 1


def _split_excess_waits(nc, lim=_MAX_WAITS):
    """Hardware engine instructions have a bounded number of sync-wait slots.
    Tile can attach more after wide cross-engine fan-ins (walrus then fails
    with 'Too many sync wait commands').  Move excess waits onto same-engine
    NOPs inserted immediately before the offending instruction — queue order
    on the engine preserves semantics."""
    nid = [0]
    for bb in nc.main_func.blocks:
        insts = list(bb.instructions)
        out = []
        changed = False
        for ins in insts:
            si = ins.sync_info
            ilim = 2 if isinstance(ins, mybir.InstEventSemaphore) else lim
            if si is not None and si.on_wait and len(si.on_wait) > ilim:
                waits = list(si.on_wait)
                head, tail = waits[:-ilim], waits[-ilim:]
                while head:
                    chunk, head = head[:lim], head[lim:]
                    nop = mybir.InstNoOp(name=f"waitnop-{nid[0]}", ins=[],
                                         outs=[], engine=ins.engine)
                    nid[0] += 1
                    nop.sync_info = mybir.SyncInfo(on_wait=chunk, on_update=[])
                    out.append(nop)
                ins.sync_info = mybir.SyncInfo(on_wait=tail,
                                              on_update=list(si.on_update))
                changed = True
            out.append(ins)
        if changed:
            bb.instructions = out


def build_nc():
    nc = bass.Bass()

    x0 = nc.dram_tensor("x0", [27, 2, 224, 224], BF16, kind="ExternalInput")
    wb = nc.dram_tensor("wb", [128, WB_COLS], BF16, kind="ExternalInput")
    bb = nc.dram_tensor("bb", [128, BB_COLS], F32, kind="ExternalInput")
    eout = nc.dram_tensor("eout", [128, 16], F32, kind="ExternalOutput")

    # DRAM scratch fmaps (padded, bf16)
    fmaps = []          # conv outputs per layer
    pooled = {}         # pooled outputs per pooling layer index
    for li, L in enumerate(LAYERS):
        H = L["H"]
        fmaps.append(nc.dram_tensor(f"f{li}", [L["Cout"], 2, H + 2, H + 2], BF16))
        if L["pool"]:
            pooled[li] = nc.dram_tensor(f"p{li}",
                                        [L["Cout"], 2, H // 2 + 2, H // 2 + 2],
                                        BF16)

    with tile.TileContext(nc) as tc:
        import contextlib
        with contextlib.ExitStack() as top:
            tc.wpool = top.enter_context(tc.tile_pool(name="w", bufs=6))
            const_pool = top.enter_context(tc.tile_pool(name="const", bufs=1))
            # top-level co-resident energy pools: energy work overlaps the
            # conv layers that follow its producing layer
            ep = {
                "f": top.enter_context(tc.tile_pool(name="ef", bufs=2)),
                "r": top.enter_context(tc.tile_pool(name="er", bufs=2)),
                "t": top.enter_context(tc.tile_pool(name="et", bufs=2)),
                "s": top.enter_context(tc.tile_pool(name="es", bufs=1)),
                "a": top.enter_context(tc.tile_pool(name="ea", bufs=4)),
            }

            bias_sb = const_pool.tile([128, BB_COLS], F32)
            nc.sync.dma_start(bias_sb, bb[:, :])
            zrow = const_pool.tile([128, 2, 226], BF16)
            nc.gpsimd.memset(zrow, 0.0)
            res = const_pool.tile([128, 16], F32)
            nc.vector.memset(res, 0.0)

            src = x0
            energy_fmaps = {}
            for li, L in enumerate(LAYERS):
                dst = fmaps[li]
                pl = pooled.get(li)
                with nc.named_scope(f"conv{li}"):
                    _emit_conv_layer(tc, nc, li, L, src, dst, pl, wb, bias_sb,
                                     zrow)
                if L["energy"] is not None:
                    energy_fmaps[L["energy"]] = dst
                src = pl if L["pool"] else dst

            # energy passes emitted LAST: lower scheduler priority, so the
            # conv chain keeps the contended engines and energy fills idle
            # slots (each band is dep-ready as soon as its fmap rows land)
            for si in range(5):
                with nc.named_scope(f"energy{si}"):
                    _emit_energy(tc, nc, si, energy_fmaps[si], res, ep)

            nc.sync.dma_start(eout[:, :], res)

    _split_excess_waits(nc)
    return nc


# ---------------------------------------------------------------------------
# host-side input prep / output decode
# ---------------------------------------------------------------------------

def _prep_weights(params):
    WB = np.zeros((128, WB_COLS), np.float32)
    BB = np.zeros((128, BB_COLS), np.float32)
    for li, L in enumerate(LAYERS):
        w = np.asarray(params[li][0], np.float32)   # [Cout, Cin, 3, 3]
        b = np.asarray(params[li][1], np.float32)   # [Cout]
        Cout = L["Cout"]
        GK = 32 if L["K"] < 64 else 64
        if li == 0:
            # im2row: row = (3*dy+dx)*3 + c, single tap, replicated per group
            blk = np.transpose(w, (2, 3, 1, 0)).reshape(27, Cout)
            for g in range(L["G"]):
                WB[g * GK:g * GK + 27, L["wcol"]:L["wcol"] + Cout] = blk
        else:
            Cin, taps = L["Cin"], L["taps"]
            for ci in range(L["ci_tiles"]):
                K = min(128, Cin - ci * 128)
                blk = np.transpose(w[:, ci * 128:ci * 128 + K], (1, 2, 3, 0))
                blk = blk.reshape(K, taps * Cout)  # [k, tap*Cout]
                c0 = L["wcol"] + ci * taps * Cout
                if L["G"] > 1:
                    for g in range(L["G"]):
                        WB[g * GK:g * GK + K, c0:c0 + taps * Cout] = blk
                else:
                    WB[0:K, c0:c0 + taps * Cout] = blk
        for co in range(L["co_tiles"]):
            M = min(128, Cout - co * 128)
            BB[0:M, L["bcol"] + co] = b[co * 128:co * 128 + M]
    return WB.astype(NPBF16), BB


def _prep_im2row(img_pair):
    """img_pair: [2, 3, 224, 224] fp32 -> [27, 2, 224, 224] bf16."""
    xp = np.zeros((2, 3, 226, 226), np.float32)
    xp[:, :, 1:225, 1:225] = img_pair
    A = np.empty((27, 2, 224, 224), np.float32)
    for t in range(9):
        dy, dx = divmod(t, 3)
        for c in range(3):
            A[t * 3 + c] = xp[:, c, dy:dy + 224, dx:dx + 224]
    return A.astype(NPBF16)


def _decode(res_list):
    """res_list: per-core [128, 16] fp32 -> softmax over all 16 images."""
    scores = np.zeros(16, np.float64)
    for k, res in enumerate(res_list):
        res = np.asarray(res, np.float64)
        for i_loc in range(2):
            gimg = 2 * k + i_loc
            e = []
            for si in range(5):
                C, H = _SCALE_C[si], _SCALE_H[si]
                cols = _SCALE_COLS[si]
                if len(cols) == 1:
                    s = res[64 * i_loc:64 * (i_loc + 1), cols[0]].sum()
                else:
                    s = res[:, cols[i_loc]].sum()
                e.append(s / (64.0 * C * H * H))
            scores[gimg] = np.mean(e)
    # image order: first 8 are S1, next 8 are S2 (cores hold [2k, 2k+1])
    z = scores - scores.max()
    p = np.exp(z)
    p /= p.sum()
    return p.astype(np.float32)


_NC_CACHE = {}
LAST_RESULT = None
# all conv biases in this problem are zero (setup_inputs hardcodes them);
# checked at kernel() time — enables the DVE relu routing
_ZERO_BIAS = [True]


def kernel(S1_VGG_in, S2_VGG_in, params):
    global LAST_RESULT
    from concourse.bass_utils import run_bass_kernel_spmd

    S1 = np.asarray(S1_VGG_in, np.float32)
    S2 = np.asarray(S2_VGG_in, np.float32)
    imgs = np.concatenate([S1, S2], axis=0)      # [16, 3, 224, 224]

    WB, BB = _prep_weights(params)
    _ZERO_BIAS[0] = not np.any(BB)
    in_maps = []
    for k in range(N_CORES):
        x0 = _prep_im2row(imgs[2 * k:2 * k + 2])
        in_maps.append({"x0": x0, "wb": WB, "bb": BB})

    key = ("nc", _ZERO_BIAS[0])
    if key not in _NC_CACHE:
        _NC_CACHE[key] = build_nc()
    nc = _NC_CACHE[key]

    trace = os.environ.get("KBENCH_TRACE", "0") == "1"
    kw = {}
    if trace:
        kw = dict(trace=True, trace_cores=[0])
    r = run_bass_kernel_spmd(nc, in_maps, core_ids=list(range(N_CORES)), **kw)
    LAST_RESULT = r
    return _decode([m["eout"] for m in r.results])


if __name__ == "__main__":
    if "--dry" in sys.argv:
        t0 = time.time()
        nc = build_nc()
        n = len(nc.m.functions[0].blocks[0].instructions) if hasattr(
            nc.m.functions[0], "blocks") else -1
        print(f"build ok in {time.time()-t0:.1f}s")
        try:
            total = sum(len(bb.instructions) for bb in nc.main_func.blocks)
            print("instructions:", total)
        except Exception as e:
            print("count failed:", e)
